# revision 42
# baseline (speedup 1.0000x reference)
"""Trainium2 Bass kernel for nn_Attention_86268713108190.

7 independent attention "bands" over batch 8, n=512, d=512, 8 heads,
shared Wqkv/Wout. Sharding: data-parallel over batch — core c handles
batch index c (7 band-samples of [512, 512] each).

v2: software-pipelined band schedule built around keeping the PE
(tensor engine) continuously fed so the HAM clock gate stays at
2.4 GHz (it throttles to 1.2 GHz after ~3.4 us of idle):

  - Per band: QKV/V projections (12 groups x 4 matmuls, f32r), S^T
    matmuls per head pair as row-tiled concurrent pairs
    (tile_position (0,0)/(64,0), K=64) into a 2-bank PSUM tile, AV with a
    ones column appended to V so the softmax denominator falls out of
    the AV matmul for free (row 64).
  - The attention phase is ACT-paced (exp), so the NEXT band's
    projection matmuls and the PREVIOUS band's out-projection are
    interleaved as fillers between S units to keep PE busy.
  - Softmax normalize: AV PSUM is evacuated to SBUF ([65, 512] DVE
    copies), each pair's [1, 1024] denominator row-pair is moved by
    the idle GPSIMD engine into partition-row 32g of a shared tile, so
    the whole band's reciprocal is ONE lane-parallel Ln + ONE
    Exp(-x) — exactly 2 ACT table loads per band (Exp and Ln live in
    different first-match table sets; a load is 1283 ns, and per-pair
    [1, N] Ln/Exp ops cost 2 loads EACH because the tile scheduler
    interleaves them with softmax exps). 1/d then bounces through DRAM
    for a contiguous stride-0 broadcast DMA and 2 DVE multiplies per
    pair normalize O^T. Out-projection of band s runs as filler inside
    band s+1, so the PE never waits for the normalize chain.
  - The For_i body re-prefetches bands 0/1 x into persistent tiles at
    body end so the post-barrier start is DMA-wait-free. (An `unroll`
    option can run several software-pipelined passes per body; it
    measured timing-neutral on HW — the ~49 us/iteration intercept
    seen in an nbands sweep did not amortize — so the default is 1,
    keeping repeat semantics exact.)

Measured on HW (repeat-differenced, 8 cores concurrent): ~330 us per
7-band pass vs 635 us for the previous version on the same session
(graded baseline 551689 ns). Whole-output accuracy vs fp32 reference:
rel err ~2.9e-4 (f32r matmuls everywhere; numerics identical to v1).

Dead ends measured this session: row-major reciprocal via 4-byte
strided DMA transpose gathers (~8 us per DMA on real DGEs), 2-bank
[128, 1024] batched exps (slightly slower than split on HW), fp8/bf16
matmuls (fp8 fails the 2e-2 gate; f32r already runs 1 cycle/row at
free >= 256 so bf16 gains nothing).
"""

import contextlib
import sys

if '/opt/trn_rl_repo' not in sys.path:
    sys.path.insert(0, '/opt/trn_rl_repo')

import numpy as np

P = 128
MM_DTYPE = "f32r"
NSEQ = 512
D = 512
H = 8
DH = 64
NBANDS = 7
NCORES = 8
SCALE = D ** -0.5

_cached = None


def build_kernel(nbands=NBANDS, repeat=1, mm_dtype=MM_DTYPE, tail="free",
                 expmode="split", unroll=1):
    """The For_i body processes `unroll` full nbands-passes, software-
    pipelined across pass boundaries, so the per-iteration fixed cost
    (un-overlapped first projection, loop barrier, epilogue drain) is
    amortized. effective_passes() maps a repeat count to executed passes."""
    import concourse.mybir as mybir
    import concourse.tile as tile
    from concourse import bacc
    from concourse import library_config

    f32 = mybir.dt.float32
    f32r = (mybir.dt.float32r if mm_dtype == "f32r" else mybir.dt.bfloat16)
    Exp = mybir.ActivationFunctionType.Exp
    Ln = mybir.ActivationFunctionType.Ln
    Copy = mybir.ActivationFunctionType.Copy

    nc = bacc.Bacc("TRN2", target_bir_lowering=False, debug=False,
                   num_devices=NCORES)

    xT = nc.dram_tensor("xT", [nbands, D, NSEQ], f32r, kind="ExternalInput").ap()
    wqkvT = nc.dram_tensor("wqkvT", [D, 3 * D], f32r, kind="ExternalInput").ap()
    woutT = nc.dram_tensor("woutT", [D, D], f32r, kind="ExternalInput").ap()
    biasb = nc.dram_tensor("biasb", [P, D], f32, kind="ExternalInput").ap()
    out = nc.dram_tensor("out", [nbands, NSEQ, D], f32, kind="ExternalOutput").ap()

    nc.gpsimd.load_library(library_config.attn)

    with tile.TileContext(nc) as tc:
        with (
            tc.tile_pool(name="weights", bufs=1) as wpool,
            tc.tile_pool(name="x", bufs=2) as xpool,
            tc.tile_pool(name="qk", bufs=2) as qkpool,
            tc.tile_pool(name="v", bufs=2) as vpool,
            tc.tile_pool(name="ot", bufs=2) as otpool,
            tc.tile_pool(name="es", bufs=7) as espool,
            tc.tile_pool(name="oraw", bufs=5) as opool,
            tc.tile_pool(name="r", bufs=2) as rpool,
            tc.tile_pool(name="rb", bufs=2) as rbpool,
            tc.tile_pool(name="ob", bufs=3) as obpool,
            tc.tile_pool(name="dram", bufs=2, space="DRAM") as drampool,
            tc.tile_pool(name="psproj", bufs=2, space="PSUM") as psproj,
            tc.tile_pool(name="pss", bufs=2, space="PSUM") as pssp,
            tc.tile_pool(name="pso", bufs=2, space="PSUM") as psop,
        ):
            wq_sb = wpool.tile([P, 4, 3 * D], f32r)
            wo_sb = wpool.tile([P, 4, D], f32r)
            bias_sb = wpool.tile([P, D], f32)
            # band-end recip scratch: pair g's [1, 1024] d-row pair lives in
            # partition-row 32*g (GPSIMD cores own 16-partition slices, so
            # its writes must start on those boundaries), making the whole
            # band's reciprocal ONE lane-parallel Ln + ONE Exp (2 ACT table
            # loads per band instead of 2 per pair; a load is 1283 ns)
            dball = wpool.tile([3 * 32 + 1, 2 * NSEQ], f32)
            lgS = wpool.tile([3 * 32 + 1, 2 * NSEQ], f32)
            rccS = wpool.tile([3 * 32 + 1, 2 * NSEQ], f32)
            nc.vector.memset(dball[:], 1.0)
            # persistent x tiles for bands 0/1: prefetched at the END of the
            # previous For_i iteration so the post-barrier projections start
            # with no DMA wait
            xt0_sb = wpool.tile([P, 4, NSEQ], f32r)
            xt1_sb = wpool.tile([P, 4, NSEQ], f32r)
            wq_r = wqkvT.rearrange("(ko ki) e -> ki ko e", ki=P)
            for kt in range(4):
                nc.sync.dma_start(wq_sb[:, kt, :], wq_r[:, kt, :])
            nc.sync.dma_start(wo_sb[:], woutT.rearrange("(ko ki) e -> ki ko e", ki=P))
            nc.sync.dma_start(bias_sb[:], biasb[:])

            # per-band live tiles (keyed by band index)
            xt = {}
            qk = {}
            va = {}
            ot = {}
            oraw = {}
            es = {}
            dD = {}
            rD = {}
            drs = {}

            ET_ORDER = (0, 4, 1, 5, 2, 6, 3, 7)

            def load_x(s, into=None):
                t = (into if into is not None else
                     xpool.tile([P, 4, NSEQ], f32r, tag="xt", name="xt"))
                nc.sync.dma_start(
                    t[:],
                    xT[s % nbands].rearrange("(ko ki) n -> ki ko n", ki=P))
                xt[s] = t

            def proj_unit(s, k):
                """k in 0..7: q,k column groups; k in 8..11: v row groups."""
                if k == 0:
                    qk[s] = qkpool.tile([P, 8, NSEQ], f32r, tag="qk", name="qk")
                if k == 8:
                    va[s] = vpool.tile([P, 4, H, DH + 1], f32r, tag="va",
                                       name="va")
                if k < 8:
                    et = ET_ORDER[k]
                    ps = psproj.tile([P, NSEQ], f32, tag="psproj", name="psp")
                    for kt in range(4):
                        nc.tensor.matmul(
                            ps[:], wq_sb[:, kt, et * P:(et + 1) * P],
                            xt[s][:, kt, :], start=(kt == 0), stop=(kt == 3))
                    nc.vector.tensor_copy(qk[s][:, et, :], ps[:])
                else:
                    nt = k - 8
                    ps = psproj.tile([P, NSEQ], f32, tag="psproj", name="psp")
                    for kt in range(4):
                        nc.tensor.matmul(
                            ps[:], xt[s][:, kt, nt * P:(nt + 1) * P],
                            wq_sb[:, kt, 2 * D:3 * D],
                            start=(kt == 0), stop=(kt == 3))
                    nc.vector.tensor_copy(
                        va[s][:, nt, :, 0:DH],
                        ps[:].rearrange("p (h dh) -> p h dh", h=H))
                    ones_slice = va[s][:, nt, :, DH:DH + 1]
                    if mm_dtype == "f32r":
                        ones_slice = ones_slice.bitcast(f32)
                    nc.vector.memset(ones_slice, 1.0)

            def s_unit(s, g, jt):
                """S^T for head pair (2g, 2g+1), j-tile jt, + exps. The two
                K=64 matmuls go to distinct row-groups via tile_position (so
                they run concurrently in the PE array) into the two banks of
                one PSUM tile."""
                ps2 = pssp.tile([P, 2, NSEQ], f32, tag="pss", name="pss")
                nc.tensor.matmul(
                    ps2[:, 0, :],
                    qk[s][0:DH, 4 + g, jt * P:(jt + 1) * P],
                    qk[s][0:DH, g, :], start=True, stop=True)
                nc.tensor.matmul(
                    ps2[:, 1, :],
                    qk[s][DH:P, 4 + g, jt * P:(jt + 1) * P],
                    qk[s][DH:P, g, :], start=True, stop=True,
                    tile_position=(DH, 0))
                e = espool.tile([P, 2, NSEQ], f32r, tag="es", name="es")
                nc.scalar.activation(e[:, 0, :], ps2[:, 0, :], Exp,
                                     scale=SCALE)
                nc.scalar.activation(e[:, 1, :], ps2[:, 1, :], Exp,
                                     scale=SCALE)
                es[(g, jt)] = e

            def av_unit(s, g):
                po0 = psop.tile([DH + 1, NSEQ], f32, tag="pso", name="pso")
                po1 = psop.tile([DH + 1, NSEQ], f32, tag="pso", name="pso")
                for jt in range(4):
                    e = es.pop((g, jt))
                    nc.tensor.matmul(
                        po0[:], va[s][:, jt, 2 * g, :], e[:, 0, :],
                        start=(jt == 0), stop=(jt == 3))
                    nc.tensor.matmul(
                        po1[:], va[s][:, jt, 2 * g + 1, :], e[:, 1, :],
                        start=(jt == 0), stop=(jt == 3))
                return po0, po1

            def evac_unit(s, g, po0, po1):
                """PSUM -> SBUF (rows 0..64 incl. denominator row), then kick
                off this pair's reciprocal chain (tail-variant dependent)."""
                if tail == "none":
                    nc.scalar.activation(ot[s][0:DH, g, :], po0[0:DH, :], Copy)
                    nc.vector.tensor_copy(ot[s][DH:P, g, :], po1[0:DH, :])
                    return
                o = opool.tile([P, 2, NSEQ], f32, tag="oraw", name="oraw")
                nc.vector.tensor_copy(o[0:DH + 1, 0, :], po0[:])
                nc.vector.tensor_copy(o[0:DH + 1, 1, :], po1[:])
                oraw[(s, g)] = o
                if tail == "free":
                    collect_d(s, g)
                if tail == "row":
                    nc.sync.dma_start(dD[s][2 * g, :], o[DH:DH + 1, 0, :])
                    nc.sync.dma_start(dD[s][2 * g + 1, :], o[DH:DH + 1, 1, :])

            def recip_unit(s):
                """tail == "row" only: all 8 heads' 1/denominator,
                lane-parallel: gather the 8 [512] rows as [128, 8, 4],
                1/d = exp(-ln d), scatter back."""
                if tail != "row":
                    return
                dsb = rpool.tile([P, 8, 4], f32, tag="dsb", name="dsb")
                nc.sync.dma_start(
                    dsb[:], dD[s].rearrange("h (c p) -> p h c", p=P))
                lgT = rpool.tile([P, 32], f32, tag="lgT", name="lgT")
                nc.scalar.activation(
                    lgT[:], dsb[:].rearrange("p a b -> p (a b)"), Ln)
                rT = rpool.tile([P, 32], f32, tag="rT", name="rT")
                nc.scalar.activation(rT[:], lgT[:], Exp, scale=-1.0)
                nc.sync.dma_start(
                    rD[s][0].rearrange("h c p -> p (h c)"), rT[:])

            def collect_d(s, g):
                """Move pair g's [1, 1024] denominator row-pair into
                partition-row g of dball, on the otherwise-idle GPSIMD."""
                o = oraw[(s, g)]
                nc.gpsimd.tensor_copy(
                    dball[32 * g:32 * g + 1, :],
                    o[DH:DH + 1, :, :].rearrange("p a b -> p (a b)"))

            def tail_free_units(s):
                """Band-end reciprocal: one [4, 1024] Ln + one Exp(-x)."""
                nc.scalar.activation(lgS[:], dball[:], Ln)
                nc.scalar.activation(rccS[:], lgS[:], Exp, scale=-1.0)
                for g in range(4):
                    dr = drampool.tile([1, 2 * NSEQ], f32, tag="dr", name="dr",
                                       bufs=6)
                    nc.sync.dma_start(dr[:], rccS[32 * g:32 * g + 1, :])
                    drs[(s, g)] = dr

            def mult_unit(s, g):
                """ot[:, g, :] = O^T * (1/d) via stride-0 DRAM broadcast."""
                if tail == "none":
                    return
                rb = rbpool.tile([DH, 2 * NSEQ], f32, tag="rb", name="rb")
                if tail == "row":
                    src = rD[s][0:1, 2 * g:2 * g + 2, :, :].rearrange(
                        "o h c p -> o (h c p)")
                    nc.sync.dma_start(rb[:], src.to_broadcast((DH, 2 * NSEQ)))
                else:
                    dr = drs.pop((s, g))
                    nc.sync.dma_start(rb[:],
                                      dr[:].to_broadcast((DH, 2 * NSEQ)))
                o = oraw.pop((s, g))
                nc.vector.tensor_mul(ot[s][0:DH, g, :], o[0:DH, 0, :],
                                     rb[:, 0:NSEQ])
                nc.vector.tensor_mul(ot[s][DH:P, g, :], o[0:DH, 1, :],
                                     rb[:, NSEQ:2 * NSEQ])

            def oproj_unit(s, n):
                ps = psproj.tile([P, NSEQ], f32, tag="psproj", name="psp")
                for kt in range(4):
                    nc.tensor.matmul(
                        ps[:], ot[s][:, kt, n * P:(n + 1) * P], wo_sb[:, kt, :],
                        start=(kt == 0), stop=(kt == 3))
                ob = obpool.tile([P, D], f32, tag="ob", name="ob")
                nc.vector.tensor_add(ob[:], ps[:], bias_sb[:])
                nc.sync.dma_start(
                    out[s % nbands].rearrange(
                        "(no ni) e -> ni no e", ni=P)[:, n, :],
                    ob[:])

            n_iters = max(1, repeat // unroll)
            rep_ctx = (tc.For_i(0, n_iters, 1,
                                hint_engines=(mybir.EngineType.PE,
                                              mybir.EngineType.Activation,
                                              mybir.EngineType.DVE))
                       if n_iters > 1 else contextlib.nullcontext())
            NB = nbands * unroll  # virtual bands per For_i body
            load_x(0, into=xt0_sb)
            load_x(1, into=xt1_sb)
            with rep_ctx:
                xt[0] = xt0_sb
                xt[1] = xt1_sb
                for k in range(12):
                    proj_unit(0, k)

                for s in range(NB):
                    if s + 2 < NB:
                        load_x(s + 2)
                    if tail == "row":
                        dD[s] = drampool.tile([H, NSEQ], f32, tag="dD",
                                              name="dD")
                        rD[s] = drampool.tile([1, H, 4, P], f32, tag="rD",
                                              name="rD")
                    ot[s] = otpool.tile([P, 4, NSEQ], f32r, tag="ot",
                                        name="ot")

                    fillers = []
                    if s + 1 < NB:
                        fillers += [(proj_unit, (s + 1, k)) for k in range(12)]
                    if s >= 1:
                        fillers += [(oproj_unit, (s - 1, n)) for n in range(4)]
                    fq = iter(fillers)

                    def filler():
                        u = next(fq, None)
                        if u is not None:
                            u[0](*u[1])

                    for g in range(4):
                        for jt in range(4):
                            s_unit(s, g, jt)
                            filler()
                        if g >= 1:
                            po0, po1 = av_unit(s, g - 1)
                            evac_unit(s, g - 1, po0, po1)
                    po0, po1 = av_unit(s, 3)
                    evac_unit(s, 3, po0, po1)
                    # drain unused fillers (first and last virtual bands)
                    for u in fq:
                        u[0](*u[1])
                    if tail == "free":
                        tail_free_units(s)
                    recip_unit(s)
                    for g in range(4):
                        mult_unit(s, g)
                    xt.pop(s, None)

                for n in range(4):
                    oproj_unit(NB - 1, n)
                # prefetch bands 0/1 x for the next iteration
                load_x(0, into=xt0_sb)
                load_x(1, into=xt1_sb)
                qk.clear(); va.clear(); ot.clear()
                dD.clear(); rD.clear(); xt.clear()

    nc.compile()
    return nc


def effective_passes(repeat, unroll=1):
    """Number of full nbands-passes a build_kernel(repeat=...) executes."""
    return max(1, repeat // unroll) * unroll


def _get_nc():
    global _cached
    if _cached is None:
        _cached = build_kernel()
    return _cached


def make_in_maps(x, x_delta, x_theta, x_alpha, x_beta, x_gamma, x_upper,
                 Wqkv, Wout, bout, mm_dtype=MM_DTYPE):
    if mm_dtype == "f32r":
        cast_dt = np.float32
    else:
        import ml_dtypes
        cast_dt = ml_dtypes.bfloat16
    xs = np.stack([np.asarray(a, dtype=np.float32) for a in
                   (x, x_delta, x_theta, x_alpha, x_beta, x_gamma, x_upper)],
                  axis=0)  # [7, b, n, d]
    xsT = np.ascontiguousarray(xs.transpose(1, 0, 3, 2).astype(cast_dt))
    wqkvT = np.ascontiguousarray(np.asarray(Wqkv, np.float32).T.astype(cast_dt))
    woutT = np.ascontiguousarray(np.asarray(Wout, np.float32).T.astype(cast_dt))
    biasb = np.ascontiguousarray(
        np.broadcast_to(np.asarray(bout, np.float32)[None, :], (P, D)))
    return [
        {"xT": xsT[c], "wqkvT": wqkvT, "woutT": woutT, "biasb": biasb}
        for c in range(NCORES)
    ]


def kernel(x, x_delta, x_theta, x_alpha, x_beta, x_gamma, x_upper,
           Wqkv, Wout, bout):
    from concourse.bass_utils import run_bass_kernel_spmd

    nc = _get_nc()
    in_maps = make_in_maps(x, x_delta, x_theta, x_alpha, x_beta, x_gamma,
                           x_upper, Wqkv, Wout, bout)
    res = run_bass_kernel_spmd(nc, in_maps, core_ids=list(range(NCORES)))
    full = np.empty((NBANDS, NCORES, NSEQ, D), dtype=np.float32)
    for c in range(NCORES):
        full[:, c] = res.results[c]["out"]
    return tuple(full[i] for i in range(NBANDS))
